# revision 1
# baseline (speedup 1.0000x reference)
"""Bilateral filter (5x5 window, sigmaXY=sigmaZ=1) on 8 Trainium2 NeuronCores.

Math: with p = neighbor value, c = center value, both in [0,1):
    sim(p,c) = w_spatial * exp(-0.5(p-c)^2)
             = w_spatial * e^{-p^2/2} e^{pc} e^{-c^2/2}
The e^{-c^2/2} factor is common to num and den and cancels in the ratio.
Recentering z'=(p-1/2)(c-1/2):  e^{pc} = e^{z'} e^{p/2} e^{c/2} e^{-1/4};
e^{c/2} also cancels, e^{p/2} folds into the p-side field.  With
    t_k = e^{-(p-1/2)^2/2 + 1/8} (p-1/2)^k     (t_0(0)=1 -> zero-pad correct)
    S_k = (5x5 gaussian) (*) t_k               (banded matmuls on TensorE)
    e^{z'} ~= sum_k a_k z'^k  (degree-D weighted-LS fit on [-1/4,1/4])
    den = sum_k a_k (c-1/2)^k S_k
    M   = sum_k a_k (c-1/2)^k S_{k+1}
    out = 1/2 + M/den

Sharding: H dim split across 8 cores (64 rows each + 2-row halo, zero-padded
at image borders host-side).  Layout per core: transposed so W is the SBUF
partition dim: x[524 cols, 12 img, 68 rows]; 5 column chunks of 104 output
cols (108 input cols) fit the 128-partition limit.  The 5x5 conv = 5
dy-shifted PSUM-accumulated matmuls with a wx-banded stationary operand.

Dtypes: fields/weights/products fp16 (same DVE 2x / PE 1-cyc tiers as bf16,
8x finer mantissa), conv accumulation + S_0 + final sums fp32.  Measured
l2 rel err vs the fp32 reference: 2.9e-4 (maxrel 1.7e-3).

Engine split per chunk: TensorE 40 banded matmuls (conv), ScalarE
square/exp/PSUM-evacuations/+0.5, VectorE field chain + polynomial
assembly + reciprocal, GPSIMD the final M*(1/den) multiply, HWDGE DMAs.
"""

import numpy as np
from contextlib import ExitStack

import concourse.bass as bass
import concourse.bacc as bacc
import concourse.tile as tile
from concourse import mybir
from concourse.bass_utils import run_bass_kernel_spmd

F32 = mybir.dt.float32
BF16 = mybir.dt.float16  # fp16: same DVE/PE speed tiers as bf16, 8x finer mantissa
NP_BF16 = mybir.dt.np(BF16)

N_CORES = 8
NIMG = 12            # 4 batch * 3 channels
H = 512
W = 512
ROWS = 64            # output rows per core
R = ROWS + 4         # input rows per core incl halo
WPAD = 524           # 512 + 2+2 conv pad + 8 slack for 5*104 chunking
NCHUNK = 5
CH_OUT = 104         # output cols per chunk
CH_IN = CH_OUT + 4   # input cols per chunk
FREE_IN = NIMG * R       # 816
FREE_OUT = NIMG * ROWS   # 768
HALF_IMGS = NIMG // 2    # 6 -> matmul free n = 6*64 = 384

DEGREE = 2
GPSIMD_ADDS = False
GPSIMD_EM = False
GPS_QD = False
GPS_DEN = False
SPLIT_DMA_Q = True
ALT_XQ = True
DMA_PROLOGUE = False
U1_ACT = False
CMS_ACT = False
CONV_ORDER = (0, 1, 2, 3)
GPS_T3 = False
GPSIMD_MR = True
EM_F16 = False
POOL_BUFS = 4
ALPHA = {
    2: [1.0, 1.0096638869735923, 0.5134352510211865],
    3: [1.0, 1.0000253488679784, 0.5031493256393234, 0.1674467221730082],
}

_W1D = np.exp(-0.5 * np.array([4.0, 1.0, 0.0, 1.0, 4.0], dtype=np.float64)).astype(
    np.float32
)


def _build_bands() -> np.ndarray:
    """bands[q, dy, o] = wx[q-o] * wy[dy] for q-o in [0,4], else 0 (bf16)."""
    b = np.zeros((CH_IN, 5, CH_OUT), dtype=np.float32)
    for o in range(CH_OUT):
        for d in range(5):
            b[o + d, :, o] = _W1D[d] * _W1D
    return b.astype(NP_BF16)


def build_nc(degree: int = DEGREE, bench_iters: int = 1):
    al = ALPHA[degree]
    nord = degree + 2  # conv orders S_0..S_{degree+1}

    nc = bacc.Bacc("TRN2", target_bir_lowering=False)
    const_tensors = []
    for v in (-0.5, 0.125, 0.5):
        t_ = nc.alloc_sbuf_tensor(f"const-f32-{v}", [128, 1], F32)
        nc.const_aps.aps[(F32, v)] = t_.ap()
        const_tensors.append((t_, v))
    x_d = nc.dram_tensor("x", [WPAD, NIMG, R], BF16, kind="ExternalInput")
    b_d = nc.dram_tensor("bands", [CH_IN, 5, CH_OUT], BF16, kind="ExternalInput")
    y_d = nc.dram_tensor("y", [WPAD, NIMG, ROWS], F32, kind="ExternalOutput")

    with ExitStack() as ctx:
        tc = ctx.enter_context(tile.TileContext(nc))
        singles = ctx.enter_context(tc.tile_pool(name="singles", bufs=1))
        fields = ctx.enter_context(tc.tile_pool(name="fields", bufs=POOL_BUFS))
        evac = ctx.enter_context(tc.tile_pool(name="evac", bufs=POOL_BUFS))
        asm = ctx.enter_context(tc.tile_pool(name="asm", bufs=POOL_BUFS))
        psum = ctx.enter_context(tc.tile_pool(name="psum", bufs=1, space="PSUM"))

        for t_, v in const_tensors:
            nc.gpsimd.memset(t_.ap(), v)
        bands = singles.tile([CH_IN, 5, CH_OUT], BF16)
        # third DMA queue family (Activation-issued HWDGE) keeps the
        # gpsimd/SWDGE queue head free for the first center-copy loads
        nc.scalar.dma_start(out=bands, in_=b_d[:])

        def body():
            xts, xcs = [], []
            if DMA_PROLOGUE:
                for j in range(NCHUNK):
                    c0 = CH_OUT * j
                    x_t = fields.tile([CH_IN, NIMG, R], BF16, name="x_t",
                                      tag="x_t", bufs=NCHUNK)
                    xq = nc.gpsimd if (ALT_XQ and j % 2) else nc.sync
                    xq.dma_start(out=x_t, in_=x_d[c0 : c0 + CH_IN])
                    x_c = fields.tile([CH_OUT, NIMG, ROWS], BF16, name="x_c",
                                      tag="x_c", bufs=NCHUNK)
                    (nc.gpsimd if SPLIT_DMA_Q else nc.sync).dma_start(
                        out=x_c, in_=x_d[c0 + 2 : c0 + 2 + CH_OUT, :, 2 : 2 + ROWS]
                    )
                    xts.append(x_t); xcs.append(x_c)
            for j in range(NCHUNK):
                c0 = CH_OUT * j
                if DMA_PROLOGUE:
                    x_t, x_c = xts[j], xcs[j]
                else:
                    x_t = fields.tile([CH_IN, NIMG, R], BF16, name="x_t", tag="x_t")
                    xq = nc.gpsimd if (ALT_XQ and j % 2) else nc.sync
                    xq.dma_start(out=x_t, in_=x_d[c0 : c0 + CH_IN])
                    # center columns, partition-aligned copy (engine APs need
                    # 32-aligned base partitions, so pm[2:106] is not readable)
                    x_c = fields.tile([CH_OUT, NIMG, ROWS], BF16, name="x_c", tag="x_c")
                    (nc.gpsimd if SPLIT_DMA_Q else nc.sync).dma_start(
                        out=x_c, in_=x_d[c0 + 2 : c0 + 2 + CH_OUT, :, 2 : 2 + ROWS]
                    )

                # p-side fields (bf16, on the full padded tile incl. halo)
                sq = fields.tile([CH_IN, NIMG, R], BF16, name="sq", tag="sq")
                pm = fields.tile([CH_IN, NIMG, R], BF16, name="pm", tag="pm")
                nc.vector.tensor_scalar_add(pm, x_t, -0.5)
                if j == 0:
                    # chunk 0: square on DVE so the first exp isn't gated on
                    # both the DMA and the ACT table load
                    nc.vector.tensor_mul(sq, pm, pm)
                else:
                    nc.scalar.activation(
                        out=sq, in_=x_t, func=mybir.ActivationFunctionType.Square,
                        bias=-0.5, scale=1.0,
                    )
                t = [fields.tile([CH_IN, NIMG, R], BF16, name="t0", tag="t0")]
                nc.scalar.activation(
                    out=t[0], in_=sq, func=mybir.ActivationFunctionType.Exp,
                    bias=0.125, scale=-0.5,
                )
                # breadth-first powers: t1=t0*pm, t2=t0*sq, t3=t1*sq
                # (sq = (p-1/2)^2 is already pm^2)
                for k in range(1, nord):
                    tk = fields.tile([CH_IN, NIMG, R], BF16, name=f"t{k}", tag=f"t{k}")
                    eng = nc.gpsimd if (GPS_T3 and k == nord - 1) else nc.vector
                    if k < 2:
                        eng.tensor_mul(tk, t[k - 1], pm)
                    else:
                        eng.tensor_mul(tk, t[k - 2], sq)
                    t.append(tk)

                # 5x5 conv of each t_k on TensorE -> PSUM fp32, evac to SBUF
                s0e = evac.tile([CH_OUT, NIMG, ROWS], F32, name="s0e", tag="s0e")
                ske = [
                    evac.tile([CH_OUT, NIMG, ROWS], BF16, name=f"s{k}e", tag=f"s{k}e")
                    for k in range(1, nord)
                ]
                for k, h in [(k, h) for h in range(2) for k in CONV_ORDER]:
                    if True:
                        i0 = h * HALF_IMGS
                        sp = psum.tile([CH_OUT, HALF_IMGS, ROWS], F32, name=f"ps{k}{h}", tag=f"ps{k}{h}")
                        for dy in range(5):
                            nc.tensor.matmul(
                                sp,
                                bands[:, dy, :],
                                t[k][:, i0 : i0 + HALF_IMGS, dy : dy + ROWS],
                                start=(dy == 0),
                                stop=(dy == 4),
                            )
                        dst = s0e if k == 0 else ske[k - 1]
                        nc.scalar.copy(
                            out=dst[:, i0 : i0 + HALF_IMGS, :], in_=sp
                        )

                # Nested-form assembly (degree 2):
                #   den = S0 + u1*(S1 + b*c'*S2),  M = S1 + u1*(S2 + b*c'*S3)
                # with u1 = a1*c', b = a2/a1, c' = c-1/2.  Both scale factors
                # fold into dual-op tensor_scalar ops on the center copy.
                assert degree == 2
                u1 = asm.tile([CH_OUT, NIMG, ROWS], BF16, name="u1", tag="u1")
                if U1_ACT:
                    a1 = float(al[1])
                    nc.scalar.activation(
                        out=u1, in_=x_c,
                        func=mybir.ActivationFunctionType.Copy,
                        scale=a1, bias=-0.5 * a1,
                    )
                else:
                    nc.vector.tensor_scalar(
                        u1, x_c, -0.5, float(al[1]),
                        mybir.AluOpType.add, mybir.AluOpType.mult,
                    )
                cms = asm.tile([CH_OUT, NIMG, ROWS], BF16, name="cms", tag="cms")
                if CMS_ACT:
                    b_ = float(al[2] / al[1])
                    nc.scalar.activation(
                        out=cms, in_=x_c,
                        func=mybir.ActivationFunctionType.Copy,
                        scale=b_, bias=-0.5 * b_,
                    )
                else:
                    nc.vector.tensor_scalar(
                        cms, x_c, -0.5, float(al[2] / al[1]),
                        mybir.AluOpType.add, mybir.AluOpType.mult,
                    )

                qd = asm.tile([CH_OUT, NIMG, ROWS], BF16, name="qd", tag="qd")
                qm = asm.tile([CH_OUT, NIMG, ROWS], BF16, name="qm", tag="qm")
                den = asm.tile([CH_OUT, NIMG, ROWS], F32, name="den", tag="den")
                # em in fp16: M is a small correction (|M/den| <~ 0.5), so
                # fp16 rounding adds only ~2e-4 abs error but keeps the add
                # in the 2x DVE mode
                em = asm.tile([CH_OUT, NIMG, ROWS], BF16 if EM_F16 else F32, name="em", tag="em")
                rden = asm.tile([CH_OUT, NIMG, ROWS], F32, name="rden", tag="rden")
                mr = asm.tile([CH_OUT, NIMG, ROWS], F32, name="mr", tag="mr")
                out_t = asm.tile([CH_OUT, NIMG, ROWS], F32, name="out_t", tag="out_t")

                def corr_sum(s_lo, s_hi, q, pfx, sl):
                    """q[sl] = u1 * (s_lo + cms * s_hi) in fp16."""
                    w = asm.tile([CH_OUT, NIMG, ROWS], BF16,
                                 name=f"{pfx}w", tag=f"{pfx}w")
                    nc.vector.tensor_mul(w[sl], cms[sl], s_hi[sl])
                    x = asm.tile([CH_OUT, NIMG, ROWS], BF16,
                                 name=f"{pfx}x", tag=f"{pfx}x")
                    nc.vector.tensor_add(x[sl], s_lo[sl], w[sl])
                    nc.vector.tensor_mul(q[sl], u1[sl], x[sl])

                def assemble(sl, last):
                    corr_sum(ske[0], ske[1], qd, "d", sl)
                    corr_sum(ske[1], ske[2], qm, "m", sl)
                    deng = nc.gpsimd if (GPS_DEN and not last) else nc.vector
                    deng.tensor_add(den[sl], s0e[sl], qd[sl])
                    nc.vector.tensor_add(em[sl], ske[0][sl], qm[sl])
                    nc.vector.reciprocal_approx_fast(out=rden[sl], in_=den[sl])
                    if GPSIMD_MR and not last:
                        nc.gpsimd.tensor_mul(mr[sl], em[sl], rden[sl])
                        nc.scalar.add(out_t[sl], mr[sl], 0.5)
                    else:
                        nc.vector.tensor_mul(mr[sl], em[sl], rden[sl])
                        nc.vector.tensor_scalar_add(out_t[sl], mr[sl], 0.5)

                n_out = min(CH_OUT, W - c0)
                if j < NCHUNK - 1:
                    assemble(np.s_[:, :, :], False)
                    nc.sync.dma_start(
                        out=y_d[c0 + 2 : c0 + 2 + n_out], in_=out_t[:n_out]
                    )
                else:
                    # last chunk: per-half so the tail overlaps the final convs
                    for h in range(2):
                        i0 = h * HALF_IMGS
                        assemble(np.s_[:, i0 : i0 + HALF_IMGS, :], True)
                        nc.sync.dma_start(
                            out=y_d[c0 + 2 : c0 + 2 + n_out, i0 : i0 + HALF_IMGS],
                            in_=out_t[:n_out, i0 : i0 + HALF_IMGS],
                        )

        if bench_iters == 1:
            body()
        else:
            hints = (
                mybir.EngineType.PE,
                mybir.EngineType.DVE,
                mybir.EngineType.Activation,
                mybir.EngineType.SP,
            )
            with tc.For_i(0, bench_iters, 1, hint_engines=hints):
                body()

    nc.finalize()
    return nc


def _prep_inputs(X: np.ndarray):
    """Full X [4,3,512,512] fp32 -> per-core transposed/padded bf16 arrays."""
    Xr = np.ascontiguousarray(np.asarray(X, dtype=np.float32).reshape(NIMG, H, W))
    bands = _build_bands()
    in_maps = []
    for i in range(N_CORES):
        lo = ROWS * i - 2
        s0, s1 = max(0, lo), min(H, lo + R)
        P = np.zeros((NIMG, R, WPAD), dtype=np.float32)
        P[:, s0 - lo : s1 - lo, 2 : 2 + W] = Xr[:, s0:s1, :]
        xt = np.ascontiguousarray(P.transpose(2, 0, 1)).astype(NP_BF16)
        in_maps.append({"x": xt, "bands": bands})
    return in_maps


_NC_CACHE = {}


def kernel(X: np.ndarray) -> np.ndarray:
    key = (DEGREE, 1)
    if key not in _NC_CACHE:
        _NC_CACHE[key] = build_nc(DEGREE, 1)
    nc = _NC_CACHE[key]
    in_maps = _prep_inputs(X)
    res = run_bass_kernel_spmd(nc, in_maps, list(range(N_CORES)))
    out = np.empty((NIMG, H, W), dtype=np.float32)
    for i in range(N_CORES):
        yi = res.results[i]["y"]  # [WPAD, NIMG, ROWS]
        out[:, ROWS * i : ROWS * (i + 1), :] = yi[2 : 2 + W].transpose(1, 2, 0)
    return out.reshape(4, 3, H, W)



# revision 6
# speedup vs baseline: 1.2079x; 1.2079x over previous
"""Bilateral filter (5x5, sigmaXY=sigmaZ=1) on 8 Trainium2 NeuronCores.

Math (p neighbor, c center, both in [0,1)):
    sim(p,c) = w_spatial * exp(-0.5(p-c)^2)
             = w_spatial * t0(p) * e^{z} * (c-side factors that cancel in the ratio)
    with z = (p-1/2)(c-1/2) in [-1/4,1/4],  t0 = e^{-(p-1/2)^2/2 + 1/8}
Degree-1 weighted-LS fit  e^z ~= a0 + a1 z  gives (t_k = t0*(p-1/2)^k):
    den = S0 + c'*S1,   M = S1 + c'*S2,   S_k = gauss5x5 (*) t_k
    out = 1/2 + M/den            (a1/a0 = 1.00018 absorbed, error ~1e-4)

Engine split per 104-col chunk (x on partitions, (img,y) free):
  ACT   sq=Square(x-.5), t0=Exp, evac S0,S1 (PSUM->fp16 SBUF)
  Pool  t1=(x-.5)*t0 -> fp8, t2=(x-.5)*t1 -> fp8, qm=(x_c-.5)*S2psum
  PE    S0: 10 fp16 matmuls; S1,S2: 10 fp8 DoubleRow matmuls each at
        0.5 cyc/row (second k-tile = step-0 moving rows x e4m3 residual
        of the band weights -> ~fp11 weight precision for free)
  DVE   u1=x_c-.5, qd, den, Newton reciprocal (linear seed, 1 step),
        em, out' = em*(-rden)
Output fp16 y' = -(out-1/2); host computes 1/2 - y'.

Measured l2 rel err vs fp32 reference: ~4e-3 (gate 2e-2).
"""

import numpy as np
from contextlib import ExitStack

import concourse.bass as bass
import concourse.bacc as bacc
import concourse.tile as tile
from concourse import mybir
from concourse.bass import AP
from concourse.bass_utils import run_bass_kernel_spmd
import ml_dtypes

F32 = mybir.dt.float32
F16 = mybir.dt.float16
F8 = mybir.dt.float8e4
NP_F16 = np.float16
NP_F8 = ml_dtypes.float8_e4m3

N_CORES = 8
NIMG = 12            # 4 batch * 3 channels
H = 512
W = 512
ROWS = 64            # output rows per core
R = ROWS + 4         # input rows per core incl halo
WPAD = 524           # 512 + 2+2 conv pad + 8 slack for 5*104 chunking
NCHUNK = 5
CH_OUT = 104         # output cols per chunk
CH_IN = CH_OUT + 4   # input cols per chunk
M8 = 112             # fp8 stationary col count (16-aligned), 104 useful
GRP = 6              # imgs per matmul group (contiguous flat moving)
NMOV = GRP * R - 4   # 404: moving rows per fp8 matmul (incl 4*? junk cols)

DEGREE = 1           # kept for test.py compat (cache key)

# Newton seed for 1/den on den in [4.4, 8.8] (hard bounds of S0 + c'S1)
_RA, _RB = 4.4, 8.8
_NB = 2.0 / (_RA * _RB + (_RA + _RB) ** 2 / 4.0)
_NA = (_RA + _RB) * _NB

# engine-assignment flags
QM_ON_POOL = False   # qm on Pool needs an SBUF S2 (GPSIMD cannot read PSUM)
SQ_ON_ACT = True     # sq via ACT Square (else DVE pm/mul)
NEWTON2 = False      # second Newton step for 1/den

_W1D = np.exp(-0.5 * np.array([4.0, 1.0, 0.0, 1.0, 4.0], dtype=np.float64)).astype(
    np.float32
)


def _e4m3(a):
    return np.asarray(a, np.float32).astype(NP_F8).astype(np.float32)


def _build_bands16() -> np.ndarray:
    """b16[q, dy, o] = wx[q-o] * wy[dy] for q-o in [0,4], else 0 (fp16)."""
    b = np.zeros((CH_IN, 5, CH_OUT), dtype=np.float32)
    for o in range(CH_OUT):
        for d in range(5):
            b[o + d, :, o] = _W1D[d] * _W1D
    return b.astype(NP_F16)


def _build_bands8() -> np.ndarray:
    """b8[q, dy, kt, o]: e4m3 band + e4m3 residual in the second k-tile."""
    b = np.zeros((CH_IN, 5, 2, M8), dtype=np.float32)
    for o in range(CH_OUT):
        for d in range(5):
            for dy in range(5):
                w = np.float32(_W1D[d] * _W1D[dy])
                w0 = _e4m3(w)
                b[o + d, dy, 0, o] = w0
                b[o + d, dy, 1, o] = _e4m3(w - w0)
    return b.astype(NP_F8)


def build_nc(degree: int = DEGREE, bench_iters: int = 1):
    nc = bacc.Bacc("TRN2", target_bir_lowering=False)
    const_tensors = []
    for v in (-0.5, 0.125, 2.0, 1.0, 0.0):
        t_ = nc.alloc_sbuf_tensor(f"const-f32-{v}", [128, 1], F32)
        nc.const_aps.aps[(F32, v)] = t_.ap()
        const_tensors.append((t_, v))
    x_d = nc.dram_tensor("x", [WPAD, NIMG, R], F16, kind="ExternalInput")
    b16_d = nc.dram_tensor("b16", [CH_IN, 5, CH_OUT], F16, kind="ExternalInput")
    b8_d = nc.dram_tensor("b8", [CH_IN, 5, 2, M8], F8, kind="ExternalInput")
    y_d = nc.dram_tensor("y", [WPAD, NIMG, ROWS], F16, kind="ExternalOutput")

    AOP = mybir.AluOpType

    with ExitStack() as ctx:
        tc = ctx.enter_context(tile.TileContext(nc))
        singles = ctx.enter_context(tc.tile_pool(name="singles", bufs=1))
        fields = ctx.enter_context(tc.tile_pool(name="fields", bufs=3))
        evac = ctx.enter_context(tc.tile_pool(name="evac", bufs=2))
        asm = ctx.enter_context(tc.tile_pool(name="asm", bufs=2))
        psum = ctx.enter_context(tc.tile_pool(name="psum", bufs=1, space="PSUM"))

        for t_, v in const_tensors:
            nc.gpsimd.memset(t_.ap(), v)
        b16 = singles.tile([CH_IN, 5, CH_OUT], F16)
        b8 = singles.tile([CH_IN, 5, 2, M8], F8)
        nc.scalar.dma_start(out=b16, in_=b16_d[:])
        nc.scalar.dma_start(out=b8, in_=b8_d[:])

        def mov8(t, g, dy):
            """[108, 2(step 0), 404] moving AP into field tile t at group g, dy."""
            full = t[:]
            ap0 = [list(d) for d in full.ap][0]
            off = full.offset + g * (GRP * R) + dy
            return AP(full.tensor, off, [ap0, [0, 2], [1, NMOV]])

        def psum_view(pt):
            """[104, 2, 6, 64] useful-col view of fp8-conv psum [112, 2, 512]."""
            full = pt[:]
            ap0 = [list(d) for d in full.ap][0]
            ap0 = [ap0[0], CH_OUT]
            return AP(full.tensor, full.offset, [ap0, [512, 2], [R, GRP], [1, ROWS]])

        def body():
            for j in range(NCHUNK):
                c0 = CH_OUT * j
                n_out = min(CH_OUT, W - c0)
                x_t = fields.tile([CH_IN, NIMG, R], F16, name="x_t", tag="x_t")
                nc.sync.dma_start(out=x_t, in_=x_d[c0 : c0 + CH_IN])
                x_c = fields.tile([CH_OUT, 2, GRP, ROWS], F16, name="x_c", tag="x_c")
                nc.sync.dma_start(
                    out=x_c, in_=x_d[c0 + 2 : c0 + 2 + CH_OUT, :, 2 : 2 + ROWS]
                )

                # fields
                sq = fields.tile([CH_IN, NIMG, R], F16, name="sq", tag="sq")
                if SQ_ON_ACT:
                    nc.scalar.activation(
                        out=sq, in_=x_t, func=mybir.ActivationFunctionType.Square,
                        bias=-0.5, scale=1.0,
                    )
                else:
                    pm = fields.tile([CH_IN, NIMG, R], F16, name="pm", tag="pm")
                    nc.vector.tensor_scalar_add(pm, x_t, -0.5)
                    nc.vector.tensor_mul(sq, pm, pm)
                t0 = fields.tile([CH_IN, NIMG, R], F16, name="t0", tag="t0")
                nc.scalar.activation(
                    out=t0, in_=sq, func=mybir.ActivationFunctionType.Exp,
                    bias=0.125, scale=-0.5,
                )
                pm = fields.tile([CH_IN, NIMG, R], F16, name="pm", tag="pm")
                nc.vector.tensor_scalar_add(pm, x_t, -0.5)
                t1 = fields.tile([CH_IN, NIMG, R], F8, name="t1", tag="t1")
                nc.gpsimd.tensor_mul(t1, t0, pm)
                t2 = fields.tile([CH_IN, NIMG, R], F8, name="t2", tag="t2")
                nc.gpsimd.tensor_mul(t2, t0, sq)

                # convs: S0 fp16, S1/S2 fp8 DoubleRow (zero-cost residual ktile)
                ps0 = psum.tile([CH_OUT, 2, 8, ROWS], F32, name="ps0", tag="ps0")
                for g in range(2):
                    for dy in range(5):
                        nc.tensor.matmul(
                            ps0[:, g, 0:GRP, :],
                            b16[:, dy, :],
                            t0[:, GRP * g : GRP * (g + 1), dy : dy + ROWS],
                            start=(dy == 0),
                            stop=(dy == 4),
                        )
                ps12 = []
                for k, tk in ((1, t1), (2, t2)):
                    pt = psum.tile([M8, 2, 512], F32, name=f"ps{k}", tag=f"ps{k}")
                    for g in range(2):
                        for dy in range(5):
                            nc.tensor.matmul(
                                pt[:, g, 0:NMOV],
                                b8[:, dy, :, :],
                                mov8(tk, g, dy),
                                start=(dy == 0),
                                stop=(dy == 4),
                                perf_mode=mybir.MatmulPerfMode.DoubleRow,
                            )
                    ps12.append(pt)

                # evac + assembly
                sh = [CH_OUT, 2, GRP, ROWS]
                s0e = evac.tile(sh, F16, name="s0e", tag="s0e")
                nc.scalar.copy(out=s0e, in_=ps0[:, :, 0:GRP, :])
                s1e = evac.tile(sh, F16, name="s1e", tag="s1e")
                nc.scalar.copy(out=s1e, in_=psum_view(ps12[0]))

                u1 = asm.tile(sh, F16, name="u1", tag="u1")
                nc.vector.tensor_scalar_add(u1, x_c, -0.5)
                qd = asm.tile(sh, F16, name="qd", tag="qd")
                nc.vector.tensor_mul(qd, u1, s1e)
                den = asm.tile(sh, F16, name="den", tag="den")
                nc.vector.tensor_add(den, s0e, qd)

                # Newton: y0 = a - b*den; y1 = y0*(2 - den*y0) ~= 1/den
                y0 = asm.tile(sh, F16, name="y0", tag="y0")
                nc.vector.tensor_scalar(y0, den, -_NB, _NA, AOP.mult, AOP.add)
                tt = asm.tile(sh, F16, name="tt", tag="tt")
                nc.vector.tensor_mul(tt, den, y0)
                w2 = asm.tile(sh, F16, name="w2", tag="w2")
                nc.vector.tensor_scalar(w2, tt, 2.0, -1.0, AOP.subtract, AOP.mult)
                y1 = asm.tile(sh, F16, name="y1", tag="y1")
                nc.vector.tensor_mul(y1, w2, y0)
                if NEWTON2:
                    t2_ = asm.tile(sh, F16, name="tt2", tag="tt2")
                    nc.vector.tensor_mul(t2_, den, y1)
                    w3 = asm.tile(sh, F16, name="w3", tag="w3")
                    nc.vector.tensor_scalar(w3, t2_, 2.0, -1.0, AOP.subtract, AOP.mult)
                    y2 = asm.tile(sh, F16, name="y2", tag="y2")
                    nc.vector.tensor_mul(y2, w3, y1)
                    y1 = y2

                qm = asm.tile(sh, F16, name="qm", tag="qm")
                if QM_ON_POOL:
                    nc.gpsimd.scalar_tensor_tensor(
                        qm, x_c, 0.5, psum_view(ps12[1]), AOP.subtract, AOP.mult
                    )
                else:
                    nc.vector.tensor_mul(qm, u1, psum_view(ps12[1]))
                em = asm.tile(sh, F16, name="em", tag="em")
                nc.vector.tensor_add(em, s1e, qm)
                outm = asm.tile(sh, F16, name="outm", tag="outm")
                nc.vector.tensor_mul(outm, em, y1)

                nc.sync.dma_start(
                    out=y_d[c0 + 2 : c0 + 2 + n_out], in_=outm[:n_out]
                )

        if bench_iters == 1:
            body()
        else:
            hints = (
                mybir.EngineType.PE,
                mybir.EngineType.DVE,
                mybir.EngineType.Activation,
                mybir.EngineType.SP,
            )
            with tc.For_i(0, bench_iters, 1, hint_engines=hints):
                body()

    nc.finalize()
    return nc


def _prep_inputs(X: np.ndarray):
    """Full X [4,3,512,512] fp32 -> per-core transposed/padded fp16 arrays."""
    Xr = np.ascontiguousarray(np.asarray(X, dtype=np.float32).reshape(NIMG, H, W))
    b16 = _build_bands16()
    b8 = _build_bands8()
    in_maps = []
    for i in range(N_CORES):
        lo = ROWS * i - 2
        s0, s1 = max(0, lo), min(H, lo + R)
        P = np.zeros((NIMG, R, WPAD), dtype=np.float32)
        P[:, s0 - lo : s1 - lo, 2 : 2 + W] = Xr[:, s0:s1, :]
        xt = np.ascontiguousarray(P.transpose(2, 0, 1)).astype(NP_F16)
        in_maps.append({"x": xt, "b16": b16, "b8": b8})
    return in_maps


_NC_CACHE = {}


def kernel(X: np.ndarray) -> np.ndarray:
    key = (DEGREE, 1)
    if key not in _NC_CACHE:
        _NC_CACHE[key] = build_nc(DEGREE, 1)
    nc = _NC_CACHE[key]
    in_maps = _prep_inputs(X)
    res = run_bass_kernel_spmd(nc, in_maps, list(range(N_CORES)))
    out = np.empty((NIMG, H, W), dtype=np.float32)
    for i in range(N_CORES):
        yi = np.asarray(res.results[i]["y"], dtype=np.float32)  # [WPAD, NIMG, ROWS]
        out[:, ROWS * i : ROWS * (i + 1), :] = 0.5 + yi[2 : 2 + W].transpose(1, 2, 0)
    return out.reshape(4, 3, H, W)


# revision 9
# speedup vs baseline: 1.2511x; 1.0358x over previous
"""Bilateral filter (5x5, sigmaXY=sigmaZ=1) on 8 Trainium2 NeuronCores.

Math (p neighbor, c center, both in [0,1)):
    sim(p,c) = w_spatial * exp(-0.5(p-c)^2)
             = w_spatial * t0(p) * e^{z} * (c-side factors that cancel in the ratio)
    with z = (p-1/2)(c-1/2) in [-1/4,1/4],  t0 = e^{-(p-1/2)^2/2 + 1/8}
Degree-1 weighted-LS fit  e^z ~= a0 + a1 z  gives (t_k = t0*(p-1/2)^k):
    den = S0 + c'*S1,   M = S1 + c'*S2,   S_k = gauss5x5 (*) t_k
    out = 1/2 + M/den            (a1/a0 = 1.00018 absorbed, error ~1e-4)

Engine split per 104-col chunk (x on partitions, (img,y) free):
  ACT   sq=Square(x-.5), t0=Exp, evac S0,S1 (PSUM->fp16 SBUF)
  Pool  t1=(x-.5)*t0 -> fp8, t2=(x-.5)*t1 -> fp8, qm=(x_c-.5)*S2psum
  PE    S0: 10 fp16 matmuls; S1,S2: 10 fp8 DoubleRow matmuls each at
        0.5 cyc/row (second k-tile = step-0 moving rows x e4m3 residual
        of the band weights -> ~fp11 weight precision for free)
  DVE   u1=x_c-.5, qd, den, Newton reciprocal (linear seed, 1 step),
        em, out' = em*(-rden)
Output fp16 y' = -(out-1/2); host computes 1/2 - y'.

Measured l2 rel err vs fp32 reference: ~4e-3 (gate 2e-2).
"""

import numpy as np
from contextlib import ExitStack

import concourse.bass as bass
import concourse.bacc as bacc
import concourse.tile as tile
from concourse import mybir
from concourse.bass import AP
from concourse.bass_utils import run_bass_kernel_spmd
import ml_dtypes

F32 = mybir.dt.float32
F16 = mybir.dt.float16
F8 = mybir.dt.float8e4
NP_F16 = np.float16
NP_F8 = ml_dtypes.float8_e4m3

N_CORES = 8
NIMG = 12            # 4 batch * 3 channels
H = 512
W = 512
ROWS = 64            # output rows per core
R = ROWS + 4         # input rows per core incl halo
WPAD = 524           # 512 + 2+2 conv pad + 8 slack for 5*104 chunking
NCHUNK = 5
CH_OUT = 104         # output cols per chunk
CH_IN = CH_OUT + 4   # input cols per chunk
M8 = 112             # fp8 stationary col count (16-aligned), 104 useful
GRP = 6              # imgs per matmul group (contiguous flat moving)
NMOV = GRP * R - 4   # 404: moving rows per fp8 matmul (incl 4*? junk cols)

DEGREE = 1           # kept for test.py compat (cache key)

# Newton seed for 1/den on den in [4.4, 8.8] (hard bounds of S0 + c'S1)
_RA, _RB = 4.4, 8.8
_NB = 2.0 / (_RA * _RB + (_RA + _RB) ** 2 / 4.0)
_NA = (_RA + _RB) * _NB

# engine-assignment flags
QM_ON_POOL = False   # qm on Pool needs an SBUF S2 (GPSIMD cannot read PSUM)
QM_EVAC = True       # evacuate S2 on ACT, qm as fp16 TT on DVE (balance DVE/ACT)
PREFETCH = 2         # chunks of input DMAs issued ahead of the store queue
SQ_ON_ACT = True     # sq via ACT Square (else DVE pm/mul)
NEWTON2 = False      # second Newton step for 1/den

_W1D = np.exp(-0.5 * np.array([4.0, 1.0, 0.0, 1.0, 4.0], dtype=np.float64)).astype(
    np.float32
)


def _e4m3(a):
    return np.asarray(a, np.float32).astype(NP_F8).astype(np.float32)


def _build_bands16() -> np.ndarray:
    """b16[q, dy, o] = wx[q-o] * wy[dy] for q-o in [0,4], else 0 (fp16)."""
    b = np.zeros((CH_IN, 5, CH_OUT), dtype=np.float32)
    for o in range(CH_OUT):
        for d in range(5):
            b[o + d, :, o] = _W1D[d] * _W1D
    return b.astype(NP_F16)


def _build_bands8() -> np.ndarray:
    """b8[q, dy, kt, o]: e4m3 band + e4m3 residual in the second k-tile."""
    b = np.zeros((CH_IN, 5, 2, M8), dtype=np.float32)
    for o in range(CH_OUT):
        for d in range(5):
            for dy in range(5):
                w = np.float32(_W1D[d] * _W1D[dy])
                w0 = _e4m3(w)
                b[o + d, dy, 0, o] = w0
                b[o + d, dy, 1, o] = _e4m3(w - w0)
    return b.astype(NP_F8)


def build_nc(degree: int = DEGREE, bench_iters: int = 1):
    nc = bacc.Bacc("TRN2", target_bir_lowering=False)
    const_tensors = []
    for v in (-0.5, 0.125, 2.0, 1.0, 0.0):
        t_ = nc.alloc_sbuf_tensor(f"const-f32-{v}", [128, 1], F32)
        nc.const_aps.aps[(F32, v)] = t_.ap()
        const_tensors.append((t_, v))
    x_d = nc.dram_tensor("x", [WPAD, NIMG, R], F16, kind="ExternalInput")
    b16_d = nc.dram_tensor("b16", [CH_IN, 5, CH_OUT], F16, kind="ExternalInput")
    b8_d = nc.dram_tensor("b8", [CH_IN, 5, 2, M8], F8, kind="ExternalInput")
    y_d = nc.dram_tensor("y", [WPAD, NIMG, ROWS], F16, kind="ExternalOutput")

    AOP = mybir.AluOpType

    with ExitStack() as ctx:
        tc = ctx.enter_context(tile.TileContext(nc))
        singles = ctx.enter_context(tc.tile_pool(name="singles", bufs=1))
        fields = ctx.enter_context(tc.tile_pool(name="fields", bufs=3))
        evac = ctx.enter_context(tc.tile_pool(name="evac", bufs=2))
        asm = ctx.enter_context(tc.tile_pool(name="asm", bufs=2))
        psum = ctx.enter_context(tc.tile_pool(name="psum", bufs=1, space="PSUM"))

        for t_, v in const_tensors:
            nc.gpsimd.memset(t_.ap(), v)
        b16 = singles.tile([CH_IN, 5, CH_OUT], F16)
        b8 = singles.tile([CH_IN, 5, 2, M8], F8)
        nc.scalar.dma_start(out=b16, in_=b16_d[:])
        nc.scalar.dma_start(out=b8, in_=b8_d[:])

        def mov8(t, g, dy):
            """[108, 2(step 0), 404] moving AP into field tile t at group g, dy."""
            full = t[:]
            ap0 = [list(d) for d in full.ap][0]
            off = full.offset + g * (GRP * R) + dy
            return AP(full.tensor, off, [ap0, [0, 2], [1, NMOV]])

        def psum_view(pt):
            """[104, 2, 6, 64] useful-col view of fp8-conv psum [112, 2, 512]."""
            full = pt[:]
            ap0 = [list(d) for d in full.ap][0]
            ap0 = [ap0[0], CH_OUT]
            return AP(full.tensor, full.offset, [ap0, [512, 2], [R, GRP], [1, ROWS]])

        def body():
            xts, xcs = {}, {}

            def load(j):
                c0 = CH_OUT * j
                x_t = fields.tile([CH_IN, NIMG, R], F16, name="x_t", tag="x_t",
                                  bufs=PREFETCH + 2)
                nc.sync.dma_start(out=x_t, in_=x_d[c0 : c0 + CH_IN])
                x_c = fields.tile([CH_OUT, 2, GRP, ROWS], F16, name="x_c",
                                  tag="x_c", bufs=PREFETCH + 2)
                nc.sync.dma_start(
                    out=x_c, in_=x_d[c0 + 2 : c0 + 2 + CH_OUT, :, 2 : 2 + ROWS]
                )
                xts[j], xcs[j] = x_t, x_c

            for j in range(PREFETCH):
                load(j)
            for j in range(NCHUNK):
                c0 = CH_OUT * j
                n_out = min(CH_OUT, W - c0)
                if j + PREFETCH < NCHUNK:
                    load(j + PREFETCH)
                x_t, x_c = xts[j], xcs[j]

                # fields
                sq = fields.tile([CH_IN, NIMG, R], F16, name="sq", tag="sq")
                if SQ_ON_ACT:
                    nc.scalar.activation(
                        out=sq, in_=x_t, func=mybir.ActivationFunctionType.Square,
                        bias=-0.5, scale=1.0,
                    )
                else:
                    pm = fields.tile([CH_IN, NIMG, R], F16, name="pm", tag="pm")
                    nc.vector.tensor_scalar_add(pm, x_t, -0.5)
                    nc.vector.tensor_mul(sq, pm, pm)
                t0 = fields.tile([CH_IN, NIMG, R], F16, name="t0", tag="t0")
                nc.scalar.activation(
                    out=t0, in_=sq, func=mybir.ActivationFunctionType.Exp,
                    bias=0.125, scale=-0.5,
                )
                pm = fields.tile([CH_IN, NIMG, R], F16, name="pm", tag="pm")
                nc.vector.tensor_scalar_add(pm, x_t, -0.5)
                t1 = fields.tile([CH_IN, NIMG, R], F8, name="t1", tag="t1")
                nc.gpsimd.tensor_mul(t1, t0, pm)
                t2 = fields.tile([CH_IN, NIMG, R], F8, name="t2", tag="t2")
                nc.gpsimd.tensor_mul(t2, t0, sq)

                # convs: S0 fp16, S1/S2 fp8 DoubleRow (zero-cost residual ktile)
                ps0 = psum.tile([CH_OUT, 2, 8, ROWS], F32, name="ps0", tag="ps0")
                for g in range(2):
                    for dy in range(5):
                        nc.tensor.matmul(
                            ps0[:, g, 0:GRP, :],
                            b16[:, dy, :],
                            t0[:, GRP * g : GRP * (g + 1), dy : dy + ROWS],
                            start=(dy == 0),
                            stop=(dy == 4),
                        )
                ps12 = []
                for k, tk in ((1, t1), (2, t2)):
                    pt = psum.tile([M8, 2, 512], F32, name=f"ps{k}", tag=f"ps{k}")
                    for g in range(2):
                        for dy in range(5):
                            nc.tensor.matmul(
                                pt[:, g, 0:NMOV],
                                b8[:, dy, :, :],
                                mov8(tk, g, dy),
                                start=(dy == 0),
                                stop=(dy == 4),
                                perf_mode=mybir.MatmulPerfMode.DoubleRow,
                            )
                    ps12.append(pt)

                # evac + assembly
                sh = [CH_OUT, 2, GRP, ROWS]
                s0e = evac.tile(sh, F16, name="s0e", tag="s0e")
                nc.scalar.copy(out=s0e, in_=ps0[:, :, 0:GRP, :])
                s1e = evac.tile(sh, F16, name="s1e", tag="s1e")
                nc.scalar.copy(out=s1e, in_=psum_view(ps12[0]))

                u1 = asm.tile(sh, F16, name="u1", tag="u1")
                nc.vector.tensor_scalar_add(u1, x_c, -0.5)
                qd = asm.tile(sh, F16, name="qd", tag="qd")
                nc.vector.tensor_mul(qd, u1, s1e)
                den = asm.tile(sh, F16, name="den", tag="den")
                nc.vector.tensor_add(den, s0e, qd)

                # Newton: y0 = a - b*den; y1 = y0*(2 - den*y0) ~= 1/den
                y0 = asm.tile(sh, F16, name="y0", tag="y0")
                nc.vector.tensor_scalar(y0, den, -_NB, _NA, AOP.mult, AOP.add)
                tt = asm.tile(sh, F16, name="tt", tag="tt")
                nc.vector.tensor_mul(tt, den, y0)
                w2 = asm.tile(sh, F16, name="w2", tag="w2")
                nc.vector.tensor_scalar(w2, tt, 2.0, -1.0, AOP.subtract, AOP.mult)
                y1 = asm.tile(sh, F16, name="y1", tag="y1")
                nc.vector.tensor_mul(y1, w2, y0)
                if NEWTON2:
                    t2_ = asm.tile(sh, F16, name="tt2", tag="tt2")
                    nc.vector.tensor_mul(t2_, den, y1)
                    w3 = asm.tile(sh, F16, name="w3", tag="w3")
                    nc.vector.tensor_scalar(w3, t2_, 2.0, -1.0, AOP.subtract, AOP.mult)
                    y2 = asm.tile(sh, F16, name="y2", tag="y2")
                    nc.vector.tensor_mul(y2, w3, y1)
                    y1 = y2

                qm = asm.tile(sh, F16, name="qm", tag="qm")
                if QM_EVAC:
                    s2e = evac.tile(sh, F16, name="s2e", tag="s2e")
                    nc.scalar.copy(out=s2e, in_=psum_view(ps12[1]))
                    nc.vector.tensor_mul(qm, u1, s2e)
                else:
                    nc.vector.tensor_mul(qm, u1, psum_view(ps12[1]))
                em = asm.tile(sh, F16, name="em", tag="em")
                nc.vector.tensor_add(em, s1e, qm)
                outm = asm.tile(sh, F16, name="outm", tag="outm")
                nc.vector.tensor_mul(outm, em, y1)

                nc.sync.dma_start(
                    out=y_d[c0 + 2 : c0 + 2 + n_out], in_=outm[:n_out]
                )

        if bench_iters == 1:
            body()
        else:
            hints = (
                mybir.EngineType.PE,
                mybir.EngineType.DVE,
                mybir.EngineType.Activation,
                mybir.EngineType.SP,
            )
            with tc.For_i(0, bench_iters, 1, hint_engines=hints):
                body()

    nc.finalize()
    return nc


def _prep_inputs(X: np.ndarray):
    """Full X [4,3,512,512] fp32 -> per-core transposed/padded fp16 arrays."""
    Xr = np.ascontiguousarray(np.asarray(X, dtype=np.float32).reshape(NIMG, H, W))
    b16 = _build_bands16()
    b8 = _build_bands8()
    in_maps = []
    for i in range(N_CORES):
        lo = ROWS * i - 2
        s0, s1 = max(0, lo), min(H, lo + R)
        P = np.zeros((NIMG, R, WPAD), dtype=np.float32)
        P[:, s0 - lo : s1 - lo, 2 : 2 + W] = Xr[:, s0:s1, :]
        xt = np.ascontiguousarray(P.transpose(2, 0, 1)).astype(NP_F16)
        in_maps.append({"x": xt, "b16": b16, "b8": b8})
    return in_maps


_NC_CACHE = {}


def kernel(X: np.ndarray) -> np.ndarray:
    key = (DEGREE, 1)
    if key not in _NC_CACHE:
        _NC_CACHE[key] = build_nc(DEGREE, 1)
    nc = _NC_CACHE[key]
    in_maps = _prep_inputs(X)
    res = run_bass_kernel_spmd(nc, in_maps, list(range(N_CORES)))
    out = np.empty((NIMG, H, W), dtype=np.float32)
    for i in range(N_CORES):
        yi = np.asarray(res.results[i]["y"], dtype=np.float32)  # [WPAD, NIMG, ROWS]
        out[:, ROWS * i : ROWS * (i + 1), :] = 0.5 + yi[2 : 2 + W].transpose(1, 2, 0)
    return out.reshape(4, 3, H, W)
